# revision 1
# baseline (speedup 1.0000x reference)
"""3-layer GATv2 encoder on 8 TRN2 NeuronCores (Bass/Tile).

Sharding: nodes are assigned to cores round-robin by in-degree rank
(balanced edges + degree-uniform tiles); each core owns all edges whose
destination lands in its 12544-node range, so segment-softmax and
aggregation are core-local.  Node features / weights are replicated;
layer-boundary activations are exchanged with one AllGather.

Per-edge math per layer (heads H=2, per-head width C):
  s = xl[src] + xr[dst] + ea@We;  logit = sum_c att*lrelu(s,0.2)
  p = exp(logit) (no max-subtract: logits are O(10));  out = (sum p*xl)/sum p
Device trick: tables are scaled by |att| per channel with channels
re-ordered by (sign att_h0, sign att_h1), so logit = (sum_pos - sum_neg)
of lrelu(s_scaled) = plain strided reduces; aggregation un-scales by
1/|att| once per dst tile.
"""
import numpy as np

DBG = False
P = 128
NCORES = 8
NEG_ATT = 0.2
NEG_ACT = 0.01
MBIG = 104.0      # mask offset: exp(-104) == 0.0 in fp32
PASS_W = 32       # slots per pass (PSUM/SBUF budget)
GRP = 4           # slots per PSUM strip group


# ---------------------------------------------------------------- host prep

def _plan_graph(edge_index, N):
    src, dst = edge_index[0].astype(np.int64), edge_index[1].astype(np.int64)
    E = src.shape[0]
    npc = ((N + NCORES * P - 1) // (NCORES * P)) * P   # nodes per core
    N_pad = npc * NCORES
    deg = np.bincount(dst, minlength=N_pad)
    rank = np.argsort(-deg, kind="stable")             # node ids by degree desc
    perm = np.empty(N_pad, np.int64)                   # old -> new
    perm[rank] = (np.arange(N_pad) % NCORES) * npc + (np.arange(N_pad) // NCORES)
    inv = np.empty(N_pad, np.int64)
    inv[perm] = np.arange(N_pad)

    src_n = perm[src]
    dst_n = perm[dst]
    core = dst_n // npc
    loc = dst_n % npc
    ntiles = npc // P

    # slot index of each edge within its destination node
    order = np.argsort(dst_n, kind="stable")
    ds = dst_n[order]
    first = np.r_[True, ds[1:] != ds[:-1]]
    gstart = np.maximum.accumulate(np.where(first, np.arange(E), 0))
    slot_sorted = np.arange(E) - gstart
    slot = np.empty(E, np.int64)
    slot[order] = slot_sorted

    deg_new = np.bincount(dst_n, minlength=N_pad)
    dmax = deg_new.reshape(NCORES, ntiles, P).max(axis=(0, 2))
    d_list = np.maximum(4, ((dmax + 3) // 4) * 4).astype(np.int64)
    off = np.zeros(ntiles + 1, np.int64)
    off[1:] = np.cumsum(d_list)
    S = int(off[-1])

    IDX = np.zeros((NCORES, P, S), np.int32)
    MASK = np.zeros((NCORES, P, S), np.float32)
    EAP = np.zeros((NCORES, 10, S * P), np.float32)    # filled by caller
    colpos = off[loc // P] + slot                      # column in [0,S)
    prow = loc % P
    IDX[core, prow, colpos] = src_n.astype(np.int32)
    MASK[core, prow, colpos] = 1.0
    eacol = colpos * P + prow                          # (col-major) lhsT column
    return dict(E=E, npc=npc, N_pad=N_pad, ntiles=ntiles, d_list=d_list,
                off=off, S=S, perm=perm, inv=inv, IDX=IDX, MASK=MASK,
                EAP=EAP, core=core, eacol=eacol)


def _prep_layer(Wl, bl, Wr, br, We, att, bias, in_perm):
    """Scale/permute one layer's weights. Returns device tensors + run info."""
    H, C = att.shape
    HC = H * C
    s0 = att[0] > 0
    s1 = att[1] > 0
    key = (~s0).astype(np.int64) * 2 + (~s1).astype(np.int64)
    cho = np.argsort(key, kind="stable")               # shared within-head order
    nseg = [int(np.sum(key == k)) for k in range(4)]
    c0, c1, c2, _ = nseg
    runs = {  # (head, +1/-1) -> [(offset, length), ...] within a head's C chans
        (0, +1): [(0, c0 + c1)],
        (0, -1): [(c0 + c1, C - c0 - c1)],
        (1, +1): [(0, c0), (c0 + c1, c2)],
        (1, -1): [(c0, c1), (c0 + c1 + c2, C - c0 - c1 - c2)],
    }
    runs = {k: [(o, l) for (o, l) in v if l > 0] for k, v in runs.items()}
    colperm = np.concatenate([h * C + cho for h in range(H)])
    a = np.abs(att[:, cho]).reshape(-1).astype(np.float32)
    a = np.maximum(a, np.float32(1e-20))
    if in_perm is not None:
        Wl, Wr = Wl[in_perm], Wr[in_perm]
    Wl_aug = (np.vstack([Wl, bl[None, :]])[:, colperm] * a).astype(np.float32)
    Wr_aug = (np.vstack([Wr, br[None, :]])[:, colperm] * a).astype(np.float32)
    We_s = (We[:, colperm] * a).astype(np.float32)
    # linear-term columns: axl[:, h] = 0.2 * sum_c att*xl = 0.2 * What @ sgn_h
    sgn = np.where(att[:, cho] > 0, 1.0, -1.0).astype(np.float32)  # [H, C]
    axl_cols = np.stack([0.2 * (Wl_aug[:, h * C:(h + 1) * C] @ sgn[h])
                         for h in range(H)], 1)
    axr_cols = np.stack([0.2 * (Wr_aug[:, h * C:(h + 1) * C] @ sgn[h])
                         for h in range(H)], 1)
    Wl_aug = np.hstack([Wl_aug, axl_cols]).astype(np.float32)
    Wr_aug = np.hstack([Wr_aug, axr_cols]).astype(np.float32)
    Wea = np.stack([0.2 * (We[:, h * C:(h + 1) * C] @ att[h])
                    for h in range(H)], 1).astype(np.float32)   # [10, H]
    recipatt = np.tile((1.0 / a)[None, :], (P, 1)).astype(np.float32)
    bias_row = np.tile(bias[None, :], (P, 1)).astype(np.float32)
    return dict(H=H, C=C, HC=HC, runs=runs, cho=cho, Wl=Wl_aug, Wr=Wr_aug,
                We=We_s, Wea=Wea, recipatt=recipatt, bias_row=bias_row)


# ---------------------------------------------------------------- device

def _build(nc, g, LY, F0):
    import concourse.bass as bass
    import concourse.mybir as mybir
    import concourse.tile as tile
    from concourse.masks import make_identity
    dt = mybir.dt
    AF = mybir.ActivationFunctionType
    OP = mybir.AluOpType

    ntiles, d_list, off, S, npc = g["ntiles"], g["d_list"], g["off"], g["S"], g["npc"]
    N_pad = g["N_pad"]
    F0a = F0 + 1

    # ---- params
    t_xT = nc.declare_dram_parameter("xT", [F0a, N_pad], dt.float32, isOutput=False)
    t_xTo = nc.declare_dram_parameter("xTo", [F0a, npc], dt.float32, isOutput=False)
    t_idx = nc.declare_dram_parameter("idx", [P, S], dt.int32, isOutput=False)
    t_msk = nc.declare_dram_parameter("msk", [P, S], dt.float32, isOutput=False)
    t_ea = nc.declare_dram_parameter("eaT", [10, S * P], dt.float32, isOutput=False)
    t_aef = [nc.declare_dram_parameter(f"aef{li}", [P, S * 2], dt.float32,
                                       isOutput=False) for li in range(len(LY))]
    t_w = {}
    for li, L in enumerate(LY):
        Fa = F0a if li == 0 else 65
        for nm, arr in (("Wl", L["Wl"]), ("Wr", L["Wr"]), ("We", L["We"]),
                        ("ra", L["recipatt"]), ("bi", L["bias_row"])):
            shp = {"Wl": [Fa, L["HC"] + 2], "Wr": [Fa, L["HC"] + 2],
                   "We": [10, L["HC"]], "ra": [P, L["HC"]], "bi": [P, 64]}[nm]
            t_w[(li, nm)] = nc.declare_dram_parameter(f"{nm}{li}", shp, dt.float32,
                                                      isOutput=False)
    t_out = nc.declare_dram_parameter("o_h", [npc, 64], dt.float32, isOutput=True)
    t_dbg = {}
    if DBG:
        HC0 = LY[0]["HC"]
        for nm, shp in (("tab", [P, HC0]), ("xr", [P, HC0]), ("u0", [P, 4 * HC0]),
                        ("lg", [P, 64]), ("pex", [P, 64]), ("den", [P, 2]),
                        ("pso", [P, HC0]), ("xlg0", [P, 4 * HC0]),
                        ("s0", [P, 4 * HC0])):
            t_dbg[nm] = nc.declare_dram_parameter(f"dbg_{nm}", shp, dt.float32,
                                                  isOutput=True)

    # ---- internal dram
    xlt = [nc.dram_tensor(f"xlt{li}", [N_pad, LY[li]["HC"] + 2], dt.float32)
           for li in range(len(LY))]

    with tile.TileContext(nc) as tc:
        with (
            tc.tile_pool(name="konst", bufs=1) as kp,
            tc.tile_pool(name="ea", bufs=3) as eap,
            tc.tile_pool(name="xlg", bufs=2) as xlgp,
            tc.tile_pool(name="u", bufs=2) as up,
            tc.tile_pool(name="sm", bufs=6) as smp,
            tc.tile_pool(name="xr", bufs=2) as xrp,
            tc.tile_pool(name="tb", bufs=2) as tbp,
            tc.tile_pool(name="hh", bufs=3) as hhp,
            tc.tile_pool(name="ps_s", bufs=2, space="PSUM") as ps_s,
            tc.tile_pool(name="ps_o", bufs=2, space="PSUM") as ps_o,
            tc.tile_pool(name="ps_x", bufs=1, space="PSUM") as ps_x,
            tc.tile_pool(name="ps_t", bufs=2, space="PSUM") as ps_t,
            tc.tile_pool(name="ps_r", bufs=1, space="PSUM") as ps_r,
            tc.tile_pool(name="dram", bufs=2, space="DRAM") as drp,
        ):
            ident = kp.tile([P, P], dt.float32, tag="ident")
            make_identity(nc, ident[:])
            idx_sb = kp.tile([P, S], dt.int32, tag="idx")
            nc.sync.dma_start(idx_sb[:], t_idx[:])
            msk_sb = kp.tile([P, S], dt.float32, tag="msk")
            nc.sync.dma_start(msk_sb[:], t_msk[:])
            negM = kp.tile([P, 1], dt.float32, tag="negM")
            nc.vector.memset(negM[:], -MBIG)
            ones = kp.tile([P, 512], dt.float32, tag="ones")
            nc.vector.memset(ones[:], 1.0)
            zeros = kp.tile([P, P], dt.float32, tag="zeros")
            nc.vector.memset(zeros[:], 0.0)

            W = {}
            for (li, nm), t in t_w.items():
                shp = t.shape
                W[(li, nm)] = kp.tile(list(shp), dt.float32, tag=f"w{li}{nm}",
                                      name=f"w{li}{nm}")
                nc.sync.dma_start(W[(li, nm)][:], t[:])

            actT_l_prev = None
            actT_a_prev = None

            for li in range(len(LY)):
                L = LY[li]
                HC, C, H = L["HC"], L["C"], L["H"]
                Fa = F0a if li == 0 else 65
                last = li == len(LY) - 1

                # ---------- xl table (replicated: all N_pad rows)
                nblk = N_pad // P
                W2 = HC + 2
                bpc = max(1, 512 // W2)               # blocks per psum strip
                for b0 in range(0, nblk, bpc):
                    nb = min(bpc, nblk - b0)
                    slab = tbp.tile([Fa, bpc * P], dt.float32, tag="slab")
                    if li == 0:
                        nc.sync.dma_start(slab[:, :nb * P],
                                          t_xT[:, b0 * P:(b0 + nb) * P])
                    else:
                        cblk = 98 if npc == 12544 else (npc // P)
                        # actT_a laid out [core][65][npc]; block b -> core b//cblk
                        b_lo = b0
                        while b_lo < b0 + nb:
                            cc = b_lo // cblk
                            b_hi = min(b0 + nb, (cc + 1) * cblk)
                            nc.sync.dma_start(
                                slab[:, (b_lo - b0) * P:(b_hi - b0) * P],
                                actT_a_prev[cc, :, (b_lo - cc * cblk) * P:
                                            (b_hi - cc * cblk) * P])
                            b_lo = b_hi
                    pst = ps_t.tile([P, bpc * W2], dt.float32, tag="pst")
                    for i in range(nb):
                        nc.tensor.matmul(out=pst[:, i * W2:(i + 1) * W2],
                                         lhsT=slab[:, i * P:(i + 1) * P],
                                         rhs=W[(li, "Wl")][:],
                                         start=True, stop=True)
                    tsb = tbp.tile([P, bpc * W2], dt.float32, tag="tsb")
                    nc.scalar.activation(tsb[:, :nb * W2], pst[:, :nb * W2], AF.Copy)
                    if DBG and li == 0 and b0 == 0:
                        nc.sync.dma_start(t_dbg["tab"][:], tsb[:, :HC])
                    nc.sync.dma_start(
                        xlt[li][b0 * P:(b0 + nb) * P, :].rearrange(
                            "(b p) d -> p b d", b=nb),
                        tsb[:, :nb * W2].rearrange("p (b d) -> p b d", b=nb))

                if not last:
                    actT_l = drp.tile([65, npc], dt.float32, tag="actT_l",
                                      name=f"actT_l{li}")
                    actT_a = drp.tile([NCORES, 65, npc], dt.float32,
                                      tag="actT_a", addr_space="Shared",
                                      name=f"actT_a{li}")
                # ---------- per-tile edge pipeline
                for t in range(ntiles):
                    d_t = int(d_list[t])
                    # xr for this tile from own columns
                    xsl = xrp.tile([Fa, P], dt.float32, tag="xsl")
                    if li == 0:
                        nc.sync.dma_start(xsl[:], t_xTo[:, t * P:(t + 1) * P])
                    else:
                        nc.sync.dma_start(xsl[:],
                                          actT_l_prev[:, t * P:(t + 1) * P])
                    psx = ps_x.tile([P, W2], dt.float32, tag="psx")
                    nc.tensor.matmul(out=psx[:], lhsT=xsl[:], rhs=W[(li, "Wr")][:],
                                     start=True, stop=True)
                    xr_sb = xrp.tile([P, W2], dt.float32, tag="xr_sb")
                    nc.scalar.activation(xr_sb[:], psx[:], AF.Copy)
                    if DBG and li == 0 and t == 0:
                        nc.sync.dma_start(t_dbg["xr"][:], xr_sb[:])

                    den = smp.tile([P, 2], dt.float32, tag="den")
                    pso = ps_o.tile([P, HC], dt.float32, tag="pso")
                    npass = (d_t + PASS_W - 1) // PASS_W
                    for pi in range(npass):
                        j0 = pi * PASS_W
                        nw = min(PASS_W, d_t - j0)
                        xlg = xlgp.tile([P, PASS_W * W2], dt.float32, tag="xlg")
                        for j in range(nw):
                            nc.gpsimd.indirect_dma_start(
                                out=xlg[:, j * W2:(j + 1) * W2], out_offset=None,
                                in_=xlt[li][:],
                                in_offset=bass.IndirectOffsetOnAxis(
                                    ap=idx_sb[:, off[t] + j0 + j:off[t] + j0 + j + 1],
                                    axis=0))
                        u = up.tile([P, PASS_W * HC], dt.float32, tag="u")
                        for gg in range(0, nw, GRP):
                            ng = min(GRP, nw - gg)
                            pss = ps_s.tile([P, GRP * HC], dt.float32, tag="pss")
                            ea_sb = eap.tile([10, GRP * P], dt.float32, tag="ea_sb")
                            ccol = (off[t] + j0 + gg) * P
                            nc.sync.dma_start(ea_sb[:, :ng * P],
                                              t_ea[:, ccol:ccol + ng * P])
                            xlv = xlg[:, gg * W2:(gg + ng) * W2].rearrange(
                                "p (j w) -> p j w", w=W2)[:, :, :HC]
                            nc.tensor.matmul(
                                out=pss[:, :ng * HC], lhsT=zeros[:],
                                rhs=xlv, start=True, stop=False)
                            for j in range(ng):
                                nc.tensor.matmul(
                                    out=pss[:, j * HC:(j + 1) * HC],
                                    lhsT=ea_sb[:, j * P:(j + 1) * P],
                                    rhs=W[(li, "We")][:], start=False, stop=False)
                                nc.tensor.matmul(
                                    out=pss[:, j * HC:(j + 1) * HC],
                                    lhsT=ident[:], rhs=xr_sb[:, :HC],
                                    start=False, stop=False)
                            nc.tensor.matmul(
                                out=pss[:, :ng * HC], lhsT=ident[:],
                                rhs=xlv, start=False, stop=True)
                            if DBG and li == 0 and t == 0 and pi == 0 and gg == 0:
                                s0t = smp.tile([P, 4 * HC], dt.float32, tag="s0t",
                                               name="s0t")
                                nc.vector.tensor_copy(s0t[:, :ng * HC],
                                                      pss[:, :ng * HC])
                                nc.sync.dma_start(t_dbg["s0"][:, :ng * HC],
                                                  s0t[:, :ng * HC])
                            nc.scalar.activation(u[:, gg * HC:(gg + ng) * HC],
                                                 pss[:, :ng * HC], AF.Relu)
                        if DBG and li == 0 and t == 0 and pi == 0:
                            nc.sync.dma_start(t_dbg["u0"][:], u[:, :4 * HC])
                            nc.sync.dma_start(t_dbg["xlg0"][:], xlg[:, :4 * HC])
                        # logits: signed strided reduces over channel runs
                        lg = smp.tile([P, 2 * PASS_W], dt.float32, tag="lg")
                        for h in range(H):
                            acc = None
                            for sgn in (+1, -1):
                                for (ro, rl) in L["runs"][(h, sgn)]:
                                    red = smp.tile([P, PASS_W], dt.float32, tag="red")
                                    uv = u[:, :nw * HC].rearrange(
                                        "p (j c) -> p j c", c=HC)[:, :, h * C + ro:
                                                                  h * C + ro + rl]
                                    nc.vector.reduce_sum(red[:, :nw], uv,
                                                         axis=mybir.AxisListType.X)
                                    dstv = lg[:, h * PASS_W:h * PASS_W + nw]
                                    if acc is None:
                                        nc.vector.tensor_copy(dstv, red[:, :nw])
                                        acc = True
                                    else:
                                        nc.vector.tensor_tensor(
                                            out=dstv, in0=dstv, in1=red[:, :nw],
                                            op=OP.add if sgn > 0 else OP.subtract)
                        # linear term: lin = axl[src] + axr[dst] + aef  (x0.2
                        # already folded on host); logits = 0.8*red + lin
                        lin_t = smp.tile([P, 2 * PASS_W], dt.float32, tag="lin_t")
                        aef_t = smp.tile([P, 2 * PASS_W], dt.float32, tag="aef_t")
                        nc.sync.dma_start(
                            aef_t[:, :nw * 2],
                            t_aef[li][:, (off[t] + j0) * 2:(off[t] + j0 + nw) * 2])
                        axl_v = xlg[:, :nw * W2].rearrange(
                            "p (j w) -> p j w", w=W2)[:, :, HC:HC + 2]                             .rearrange("p j h -> p h j")
                        aef_v = aef_t[:, :nw * 2].rearrange(
                            "p (j h) -> p h j", h=2)
                        lin_v = lin_t[:].rearrange("p (h j) -> p h j", h=2)[:, :, :nw]
                        nc.vector.tensor_tensor(out=lin_v, in0=axl_v, in1=aef_v,
                                                op=OP.add)
                        axr_v = xr_sb[:, HC:HC + 2].rearrange(
                            "p (h o) -> p h o", o=1).to_broadcast([P, 2, nw])
                        nc.vector.tensor_tensor(out=lin_v, in0=lin_v, in1=axr_v,
                                                op=OP.add)
                        lgv0 = lg[:].rearrange("p (h j) -> p h j", h=2)[:, :, :nw]
                        nc.vector.scalar_tensor_tensor(
                            out=lgv0, in0=lgv0, scalar=0.8, in1=lin_v,
                            op0=OP.mult, op1=OP.add)
                        if DBG and li == 0 and t == 0 and pi == 0:
                            nc.sync.dma_start(t_dbg["lg"][:, :2 * PASS_W], lg[:])
                        # mask -> exp
                        pex = smp.tile([P, 2 * PASS_W], dt.float32, tag="pex")
                        mk = msk_sb[:, off[t] + j0:off[t] + j0 + nw]
                        mk2 = mk.rearrange("p (o j) -> p o j", o=1).to_broadcast(
                            [P, 2, nw])
                        lgv = lg[:].rearrange("p (h j) -> p h j", h=2)[:, :, :nw]
                        nc.vector.scalar_tensor_tensor(
                            out=lgv, in0=lgv, scalar=MBIG, in1=mk2,
                            op0=OP.add, op1=OP.mult)
                        pexv = pex[:].rearrange("p (h j) -> p h j", h=2)[:, :, :nw]
                        nc.scalar.activation(pexv, lgv, AF.Exp, bias=negM[:])
                        if DBG and li == 0 and t == 0 and pi == 0:
                            nc.sync.dma_start(t_dbg["pex"][:, :2 * PASS_W], pex[:])
                        # denom accumulate
                        red2 = smp.tile([P, 2], dt.float32, tag="red2")
                        nc.vector.reduce_sum(red2[:], pexv, axis=mybir.AxisListType.X)
                        if pi == 0:
                            nc.vector.tensor_copy(den[:], red2[:])
                        else:
                            nc.vector.tensor_tensor(out=den[:], in0=den[:],
                                                    in1=red2[:], op=OP.add)
                        # w = p * xlg  (in place), then aggregate per slot
                        pbc = pex[:].rearrange("p (h j) -> p h j", h=2)[:, :, :nw]
                        pbc = pbc.rearrange("p h j -> p j h")
                        pbc = pbc.rearrange("p j (h o) -> p j h o", o=1) \
                                 .to_broadcast([P, nw, 2, C])
                        xv4 = xlg[:, :nw * W2].rearrange(
                            "p (j w) -> p j w", w=W2)[:, :, :HC].rearrange(
                            "p j (h c) -> p j h c", h=2)
                        nc.vector.tensor_tensor(out=xv4, in0=xv4, in1=pbc,
                                                op=OP.mult)
                        for j in range(nw):
                            nc.tensor.matmul(out=pso[:],
                                             lhsT=ident[:],
                                             rhs=xlg[:, j * W2:j * W2 + HC],
                                             start=(pi == 0 and j == 0),
                                             stop=(pi == npass - 1 and j == nw - 1))
                    # finalize tile
                    if DBG and li == 0 and t == 0:
                        nc.sync.dma_start(t_dbg["den"][:], den[:])
                        dps = smp.tile([P, HC], dt.float32, tag="dps", name="dps")
                        nc.scalar.activation(dps[:], pso[:], AF.Copy)
                        nc.sync.dma_start(t_dbg["pso"][:], dps[:])
                    nc.vector.tensor_scalar_max(den[:], den[:], 1e-16)
                    rden = smp.tile([P, 2], dt.float32, tag="rden")
                    nc.vector.reciprocal(rden[:], den[:])
                    o1 = hhp.tile([P, HC], dt.float32, tag="o1")
                    nc.vector.tensor_tensor(out=o1[:], in0=pso[:],
                                            in1=W[(li, "ra")][:], op=OP.mult)
                    rdb = rden[:].rearrange("p (h o) -> p h o", o=1) \
                                 .to_broadcast([P, 2, C])
                    o1v = o1[:].rearrange("p (h c) -> p h c", h=2)
                    nc.vector.tensor_tensor(out=o1v, in0=o1v, in1=rdb, op=OP.mult)
                    oh = hhp.tile([P, 64], dt.float32, tag="oh")
                    if li == 0:
                        nc.vector.tensor_tensor(out=oh[:], in0=o1[:],
                                                in1=W[(li, "bi")][:], op=OP.add)
                    else:
                        # mean over heads then + bias
                        nc.vector.tensor_tensor(out=oh[:], in0=o1[:, :C],
                                                in1=o1[:, C:], op=OP.add)
                        nc.vector.tensor_scalar_mul(oh[:], oh[:], 0.5)
                        nc.vector.tensor_tensor(out=oh[:], in0=oh[:],
                                                in1=W[(li, "bi")][:], op=OP.add)
                    if last:
                        nc.sync.dma_start(t_out[t * P:(t + 1) * P, :], oh[:])
                    else:
                        hp = hhp.tile([P, 64], dt.float32, tag="hp")
                        nc.scalar.activation(hp[:], oh[:], AF.Lrelu, alpha=NEG_ACT)
                        # transpose -> actT_l
                        pstr = ps_r.tile([64, P], dt.float32, tag="pstr")
                        nc.tensor.transpose(out=pstr[:], in_=hp[:, :64],
                                            identity=ident[:])
                        trs = hhp.tile([64, P], dt.float32, tag="trs")
                        nc.scalar.activation(trs[:], pstr[:], AF.Copy)
                        nc.sync.dma_start(actT_l[0:64, t * P:(t + 1) * P], trs[:])
                if not last:
                    for q0 in range(0, npc, 512):
                        qn = min(512, npc - q0)
                        nc.sync.dma_start(actT_l[64:65, q0:q0 + qn],
                                          ones[0:1, :qn])
                    nc.gpsimd.collective_compute(
                        "AllGather", mybir.AluOpType.bypass,
                        replica_groups=[list(range(NCORES))],
                        ins=[actT_l.opt()], outs=[actT_a.opt()])
                    actT_l_prev, actT_a_prev = actT_l, actT_a
    return nc


def _run(x, edge_index, edge_attr, layers):
    import concourse.bacc as bacc
    from concourse.bass_utils import run_bass_kernel_spmd

    N, F0 = x.shape
    g = _plan_graph(edge_index, N)
    # per-core eaT in slot-column order
    eaT = edge_attr.astype(np.float32).T               # [10, E]
    for c in range(NCORES):
        m = g["core"] == c
        g["EAP"][c][:, g["eacol"][m]] = eaT[:, m]

    in_perm = None
    LY = []
    for li, Lw in enumerate(layers):
        Lp = _prep_layer(*Lw, in_perm)
        LY.append(Lp)
        if li == 0:
            in_perm = np.concatenate([Lp["cho"], Lp["C"] + Lp["cho"]])
        else:
            in_perm = Lp["cho"]
    out_perm = in_perm

    xp = np.zeros((g["N_pad"], F0), np.float32)
    xp[g["perm"][:N]] = x
    xT = np.vstack([xp.T, np.ones((1, g["N_pad"]), np.float32)])

    nc = bacc.Bacc("TRN2", target_bir_lowering=False, num_devices=NCORES)
    _build(nc, g, LY, F0)
    nc.compile()

    npc = g["npc"]
    # per-layer aef in slot order: AEF[c][p, col*2 + h]
    colpos = g["eacol"] // P
    prow = g["eacol"] % P
    AEF = []
    for li, L in enumerate(LY):
        aef_e = (edge_attr @ L["Wea"]).astype(np.float32)      # [E, 2]
        A = np.zeros((NCORES, P, g["S"] * 2), np.float32)
        for c in range(NCORES):
            m = g["core"] == c
            A[c, prow[m], colpos[m] * 2] = aef_e[m, 0]
            A[c, prow[m], colpos[m] * 2 + 1] = aef_e[m, 1]
        AEF.append(A)
    in_maps = []
    for c in range(NCORES):
        m = {"xT": xT, "xTo": np.ascontiguousarray(xT[:, c * npc:(c + 1) * npc]),
             "idx": g["IDX"][c], "msk": g["MASK"][c], "eaT": g["EAP"][c]}
        for li, L in enumerate(LY):
            m[f"Wl{li}"] = L["Wl"]; m[f"Wr{li}"] = L["Wr"]; m[f"We{li}"] = L["We"]
            m[f"ra{li}"] = L["recipatt"]; m[f"bi{li}"] = L["bias_row"]
            m[f"aef{li}"] = AEF[li][c]
        in_maps.append(m)

    import time as _time
    _t0 = _time.perf_counter()
    res = run_bass_kernel_spmd(nc, in_maps, list(range(NCORES)))
    _w1 = _time.perf_counter() - _t0
    _t0 = _time.perf_counter()
    res = run_bass_kernel_spmd(nc, in_maps, list(range(NCORES)))
    _w2 = _time.perf_counter() - _t0
    global _LAST_RES, _LAST_G, _LAST_LY, _LAST_WALL
    _LAST_RES, _LAST_G, _LAST_LY = res, g, LY
    _LAST_WALL = (_w1, _w2)
    h_new = np.concatenate([res.results[c]["o_h"] for c in range(NCORES)], axis=0)
    outp = h_new[g["perm"][:N]]                        # back to original rows
    invc = np.argsort(out_perm)                        # final column unpermute
    return np.ascontiguousarray(outp[:, invc])


def kernel(x, edge_index, edge_attr, Wl0, bl0, Wr0, br0, We0, att0, bias0,
           Wl1, bl1, Wr1, br1, We1, att1, bias1):
    x = np.asarray(x, np.float32)
    layers = [
        (np.asarray(Wl0, np.float32), np.asarray(bl0, np.float32),
         np.asarray(Wr0, np.float32), np.asarray(br0, np.float32),
         np.asarray(We0, np.float32), np.asarray(att0, np.float32),
         np.asarray(bias0, np.float32)),
    ]
    for i in range(2):
        layers.append(
            (np.asarray(Wl1[i], np.float32), np.asarray(bl1[i], np.float32),
             np.asarray(Wr1[i], np.float32), np.asarray(br1[i], np.float32),
             np.asarray(We1[i], np.float32), np.asarray(att1[i], np.float32),
             np.asarray(bias1[i], np.float32)))
    return _run(x, np.asarray(edge_index), np.asarray(edge_attr, np.float32),
                layers)



# revision 3
# speedup vs baseline: 704.1230x; 704.1230x over previous
"""3-layer GATv2 encoder on 8 TRN2 NeuronCores (Bass/Tile).

Sharding: nodes are assigned to cores round-robin by in-degree rank
(balanced edges + degree-uniform tiles); each core owns all edges whose
destination lands in its 12544-node range, so segment-softmax and
aggregation are core-local.  Node features / weights are replicated;
layer-boundary activations are exchanged with one AllGather.

Per-edge math per layer (heads H=2, per-head width C):
  s = xl[src] + xr[dst] + ea@We;  logit = sum_c att*lrelu(s,0.2)
  p = exp(logit) (no max-subtract: logits are O(10));  out = (sum p*xl)/sum p
Device trick: tables are scaled by |att| per channel with channels
re-ordered by (sign att_h0, sign att_h1), so logit = (sum_pos - sum_neg)
of lrelu(s_scaled) = plain strided reduces; aggregation un-scales by
1/|att| once per dst tile.
"""
import numpy as np

DBG = False
P = 128
NCORES = 8
NEG_ATT = 0.2
NEG_ACT = 0.01
MBIG = 104.0      # mask offset: exp(-104) == 0.0 in fp32
PASS_W = 32       # slots per pass (PSUM/SBUF budget)
GRP = 4           # slots per PSUM strip group


# ---------------------------------------------------------------- host prep

def _plan_graph(edge_index, N):
    src, dst = edge_index[0].astype(np.int64), edge_index[1].astype(np.int64)
    E = src.shape[0]
    npc = ((N + NCORES * P - 1) // (NCORES * P)) * P   # nodes per core
    N_pad = npc * NCORES
    deg = np.bincount(dst, minlength=N_pad)
    rank = np.argsort(-deg, kind="stable")             # node ids by degree desc
    perm = np.empty(N_pad, np.int64)                   # old -> new
    perm[rank] = (np.arange(N_pad) % NCORES) * npc + (np.arange(N_pad) // NCORES)
    inv = np.empty(N_pad, np.int64)
    inv[perm] = np.arange(N_pad)

    src_n = perm[src]
    dst_n = perm[dst]
    core = dst_n // npc
    loc = dst_n % npc
    ntiles = npc // P

    # slot index of each edge within its destination node
    order = np.argsort(dst_n, kind="stable")
    ds = dst_n[order]
    first = np.r_[True, ds[1:] != ds[:-1]]
    gstart = np.maximum.accumulate(np.where(first, np.arange(E), 0))
    slot_sorted = np.arange(E) - gstart
    slot = np.empty(E, np.int64)
    slot[order] = slot_sorted

    deg_new = np.bincount(dst_n, minlength=N_pad)
    dmax = deg_new.reshape(NCORES, ntiles, P).max(axis=(0, 2))
    d_list = np.maximum(4, ((dmax + 3) // 4) * 4).astype(np.int64)
    off = np.zeros(ntiles + 1, np.int64)
    off[1:] = np.cumsum(d_list)
    S = int(off[-1])

    IDX = np.zeros((NCORES, P, S), np.int32)
    MASK = np.zeros((NCORES, P, S), np.float32)
    EAP = np.zeros((NCORES, 10, S * P), np.float32)    # filled by caller
    colpos = off[loc // P] + slot                      # column in [0,S)
    prow = loc % P
    IDX[core, prow, colpos] = src_n.astype(np.int32)
    MASK[core, prow, colpos] = 1.0
    eacol = colpos * P + prow                          # (col-major) lhsT column
    return dict(E=E, npc=npc, N_pad=N_pad, ntiles=ntiles, d_list=d_list,
                off=off, S=S, perm=perm, inv=inv, IDX=IDX, MASK=MASK,
                EAP=EAP, core=core, eacol=eacol)


def _prep_layer(Wl, bl, Wr, br, We, att, bias, in_perm):
    """Scale/permute one layer's weights. Returns device tensors + run info."""
    H, C = att.shape
    HC = H * C
    s0 = att[0] > 0
    s1 = att[1] > 0
    key = (~s0).astype(np.int64) * 2 + (~s1).astype(np.int64)
    cho = np.argsort(key, kind="stable")               # shared within-head order
    nseg = [int(np.sum(key == k)) for k in range(4)]
    c0, c1, c2, _ = nseg
    runs = {  # (head, +1/-1) -> [(offset, length), ...] within a head's C chans
        (0, +1): [(0, c0 + c1)],
        (0, -1): [(c0 + c1, C - c0 - c1)],
        (1, +1): [(0, c0), (c0 + c1, c2)],
        (1, -1): [(c0, c1), (c0 + c1 + c2, C - c0 - c1 - c2)],
    }
    runs = {k: [(o, l) for (o, l) in v if l > 0] for k, v in runs.items()}
    colperm = np.concatenate([h * C + cho for h in range(H)])
    a = np.abs(att[:, cho]).reshape(-1).astype(np.float32)
    a = np.maximum(a, np.float32(1e-20))
    if in_perm is not None:
        Wl, Wr = Wl[in_perm], Wr[in_perm]
    Wl_aug = (np.vstack([Wl, bl[None, :]])[:, colperm] * a).astype(np.float32)
    Wr_aug = (np.vstack([Wr, br[None, :]])[:, colperm] * a).astype(np.float32)
    We_s = (We[:, colperm] * a).astype(np.float32)
    # linear-term columns: axl[:, h] = 0.2 * sum_c att*xl = 0.2 * What @ sgn_h
    sgn = np.where(att[:, cho] > 0, 1.0, -1.0).astype(np.float32)  # [H, C]
    axl_cols = np.stack([0.2 * (Wl_aug[:, h * C:(h + 1) * C] @ sgn[h])
                         for h in range(H)], 1)
    axr_cols = np.stack([0.2 * (Wr_aug[:, h * C:(h + 1) * C] @ sgn[h])
                         for h in range(H)], 1)
    Wl_aug = np.hstack([Wl_aug, axl_cols]).astype(np.float32)
    Wr_aug = np.hstack([Wr_aug, axr_cols]).astype(np.float32)
    Wea = np.stack([0.2 * (We[:, h * C:(h + 1) * C] @ att[h])
                    for h in range(H)], 1).astype(np.float32)   # [10, H]
    recipatt = np.tile((1.0 / a)[None, :], (P, 1)).astype(np.float32)
    bias_row = np.tile(bias[None, :], (P, 1)).astype(np.float32)
    return dict(H=H, C=C, HC=HC, runs=runs, cho=cho, Wl=Wl_aug, Wr=Wr_aug,
                We=We_s, Wea=Wea, recipatt=recipatt, bias_row=bias_row)


# ---------------------------------------------------------------- device

def _build(nc, g, LY, F0):
    import concourse.bass as bass
    import concourse.mybir as mybir
    import concourse.tile as tile
    from concourse.masks import make_identity
    dt = mybir.dt
    AF = mybir.ActivationFunctionType
    OP = mybir.AluOpType

    ntiles, d_list, off, S, npc = g["ntiles"], g["d_list"], g["off"], g["S"], g["npc"]
    N_pad = g["N_pad"]
    F0a = F0 + 1

    # ---- params
    t_xT = nc.declare_dram_parameter("xT", [F0a, N_pad], dt.float32, isOutput=False)
    t_xTo = nc.declare_dram_parameter("xTo", [F0a, npc], dt.float32, isOutput=False)
    t_idx = nc.declare_dram_parameter("idx", [P, S], dt.int32, isOutput=False)
    t_msk = nc.declare_dram_parameter("msk", [P, S], dt.float32, isOutput=False)
    t_ea = nc.declare_dram_parameter("eaT", [10, S * P], dt.float32, isOutput=False)
    t_aef = [nc.declare_dram_parameter(f"aef{li}", [P, S * 2], dt.float32,
                                       isOutput=False) for li in range(len(LY))]
    t_w = {}
    for li, L in enumerate(LY):
        Fa = F0a if li == 0 else 65
        for nm, arr in (("Wl", L["Wl"]), ("Wr", L["Wr"]), ("We", L["We"]),
                        ("ra", L["recipatt"]), ("bi", L["bias_row"])):
            shp = {"Wl": [Fa, L["HC"] + 2], "Wr": [Fa, L["HC"] + 2],
                   "We": [10, L["HC"]], "ra": [P, L["HC"]], "bi": [P, 64]}[nm]
            t_w[(li, nm)] = nc.declare_dram_parameter(f"{nm}{li}", shp, dt.float32,
                                                      isOutput=False)
    t_out = nc.declare_dram_parameter("o_h", [npc, 64], dt.float32, isOutput=True)
    t_dbg = {}
    if DBG:
        HC0 = LY[0]["HC"]
        for nm, shp in (("tab", [P, HC0]), ("xr", [P, HC0]), ("u0", [P, 4 * HC0]),
                        ("lg", [P, 64]), ("pex", [P, 64]), ("den", [P, 2]),
                        ("pso", [P, HC0]), ("xlg0", [P, 4 * HC0]),
                        ("s0", [P, 4 * HC0])):
            t_dbg[nm] = nc.declare_dram_parameter(f"dbg_{nm}", shp, dt.float32,
                                                  isOutput=True)

    # ---- internal dram
    xlt = [nc.dram_tensor(f"xlt{li}", [N_pad, LY[li]["HC"] + 2], dt.float32)
           for li in range(len(LY))]

    with tile.TileContext(nc) as tc:
        with (
            tc.tile_pool(name="konst", bufs=1) as kp,
            tc.tile_pool(name="ea", bufs=3) as eap,
            tc.tile_pool(name="xlg", bufs=2) as xlgp,
            tc.tile_pool(name="u", bufs=2) as up,
            tc.tile_pool(name="sm", bufs=6) as smp,
            tc.tile_pool(name="xr", bufs=2) as xrp,
            tc.tile_pool(name="tb", bufs=2) as tbp,
            tc.tile_pool(name="hh", bufs=3) as hhp,
            tc.tile_pool(name="ps_s", bufs=2, space="PSUM") as ps_s,
            tc.tile_pool(name="ps_o", bufs=2, space="PSUM") as ps_o,
            tc.tile_pool(name="ps_x", bufs=1, space="PSUM") as ps_x,
            tc.tile_pool(name="ps_t", bufs=2, space="PSUM") as ps_t,
            tc.tile_pool(name="ps_r", bufs=1, space="PSUM") as ps_r,
            tc.tile_pool(name="dram", bufs=2, space="DRAM") as drp,
        ):
            ident = kp.tile([P, P], dt.float32, tag="ident")
            make_identity(nc, ident[:])
            idx_sb = kp.tile([P, S], dt.int32, tag="idx")
            nc.sync.dma_start(idx_sb[:], t_idx[:])
            msk_sb = kp.tile([P, S], dt.float32, tag="msk")
            nc.sync.dma_start(msk_sb[:], t_msk[:])
            negM = kp.tile([P, 1], dt.float32, tag="negM")
            nc.vector.memset(negM[:], -MBIG)
            ones = kp.tile([P, 512], dt.float32, tag="ones")
            nc.vector.memset(ones[:], 1.0)
            zeros = kp.tile([P, P], dt.float32, tag="zeros")
            nc.vector.memset(zeros[:], 0.0)

            W = {}
            for (li, nm), t in t_w.items():
                shp = t.shape
                W[(li, nm)] = kp.tile(list(shp), dt.float32, tag=f"w{li}{nm}",
                                      name=f"w{li}{nm}")
                nc.sync.dma_start(W[(li, nm)][:], t[:])

            actT_l_prev = None
            actT_a_prev = None

            for li in range(len(LY)):
                L = LY[li]
                HC, C, H = L["HC"], L["C"], L["H"]
                Fa = F0a if li == 0 else 65
                last = li == len(LY) - 1

                # ---------- xl table (replicated: all N_pad rows)
                nblk = N_pad // P
                W2 = HC + 2
                bpc = max(1, 512 // W2)               # blocks per psum strip
                for b0 in range(0, nblk, bpc):
                    nb = min(bpc, nblk - b0)
                    slab = tbp.tile([Fa, bpc * P], dt.float32, tag="slab")
                    if li == 0:
                        nc.sync.dma_start(slab[:, :nb * P],
                                          t_xT[:, b0 * P:(b0 + nb) * P])
                    else:
                        cblk = 98 if npc == 12544 else (npc // P)
                        # actT_a laid out [core][65][npc]; block b -> core b//cblk
                        b_lo = b0
                        while b_lo < b0 + nb:
                            cc = b_lo // cblk
                            b_hi = min(b0 + nb, (cc + 1) * cblk)
                            nc.sync.dma_start(
                                slab[:, (b_lo - b0) * P:(b_hi - b0) * P],
                                actT_a_prev[cc, :, (b_lo - cc * cblk) * P:
                                            (b_hi - cc * cblk) * P])
                            b_lo = b_hi
                    pst = ps_t.tile([P, bpc * W2], dt.float32, tag="pst")
                    for i in range(nb):
                        nc.tensor.matmul(out=pst[:, i * W2:(i + 1) * W2],
                                         lhsT=slab[:, i * P:(i + 1) * P],
                                         rhs=W[(li, "Wl")][:],
                                         start=True, stop=True)
                    tsb = tbp.tile([P, bpc * W2], dt.float32, tag="tsb")
                    nc.scalar.activation(tsb[:, :nb * W2], pst[:, :nb * W2], AF.Copy)
                    if DBG and li == 0 and b0 == 0:
                        nc.sync.dma_start(t_dbg["tab"][:], tsb[:, :HC])
                    nc.sync.dma_start(
                        xlt[li][b0 * P:(b0 + nb) * P, :].rearrange(
                            "(b p) d -> p b d", b=nb),
                        tsb[:, :nb * W2].rearrange("p (b d) -> p b d", b=nb))

                if not last:
                    actT_l = drp.tile([65, npc], dt.float32, tag="actT_l",
                                      name=f"actT_l{li}")
                    actT_a = drp.tile([NCORES, 65, npc], dt.float32,
                                      tag="actT_a", addr_space="Shared",
                                      name=f"actT_a{li}")
                # ---------- per-tile edge pipeline
                for t in range(ntiles):
                    d_t = int(d_list[t])
                    # xr for this tile from own columns
                    xsl = xrp.tile([Fa, P], dt.float32, tag="xsl")
                    if li == 0:
                        nc.sync.dma_start(xsl[:], t_xTo[:, t * P:(t + 1) * P])
                    else:
                        nc.sync.dma_start(xsl[:],
                                          actT_l_prev[:, t * P:(t + 1) * P])
                    psx = ps_x.tile([P, W2], dt.float32, tag="psx")
                    nc.tensor.matmul(out=psx[:], lhsT=xsl[:], rhs=W[(li, "Wr")][:],
                                     start=True, stop=True)
                    xr_sb = xrp.tile([P, W2], dt.float32, tag="xr_sb")
                    nc.scalar.activation(xr_sb[:], psx[:], AF.Copy)
                    if DBG and li == 0 and t == 0:
                        nc.sync.dma_start(t_dbg["xr"][:], xr_sb[:])

                    den = smp.tile([P, 2], dt.float32, tag="den")
                    pso = ps_o.tile([P, HC], dt.float32, tag="pso")
                    npass = (d_t + PASS_W - 1) // PASS_W
                    for pi in range(npass):
                        j0 = pi * PASS_W
                        nw = min(PASS_W, d_t - j0)
                        xlg = xlgp.tile([P, PASS_W * W2], dt.float32, tag="xlg")
                        for j in range(nw):
                            nc.gpsimd.indirect_dma_start(
                                out=xlg[:, j * W2:(j + 1) * W2], out_offset=None,
                                in_=xlt[li][:],
                                in_offset=bass.IndirectOffsetOnAxis(
                                    ap=idx_sb[:, off[t] + j0 + j:off[t] + j0 + j + 1],
                                    axis=0))
                        u = up.tile([P, PASS_W * HC], dt.float32, tag="u")
                        for gg in range(0, nw, GRP):
                            ng = min(GRP, nw - gg)
                            pss = ps_s.tile([P, GRP * HC], dt.float32, tag="pss")
                            ea_sb = eap.tile([10, GRP * P], dt.float32, tag="ea_sb")
                            ccol = (off[t] + j0 + gg) * P
                            nc.sync.dma_start(ea_sb[:, :ng * P],
                                              t_ea[:, ccol:ccol + ng * P])
                            xlv = xlg[:, gg * W2:(gg + ng) * W2].rearrange(
                                "p (j w) -> p j w", w=W2)[:, :, :HC]
                            nc.tensor.matmul(
                                out=pss[:, :ng * HC], lhsT=zeros[:],
                                rhs=xlv, start=True, stop=False)
                            for j in range(ng):
                                nc.tensor.matmul(
                                    out=pss[:, j * HC:(j + 1) * HC],
                                    lhsT=ea_sb[:, j * P:(j + 1) * P],
                                    rhs=W[(li, "We")][:], start=False, stop=False)
                                nc.tensor.matmul(
                                    out=pss[:, j * HC:(j + 1) * HC],
                                    lhsT=ident[:], rhs=xr_sb[:, :HC],
                                    start=False, stop=False)
                            nc.tensor.matmul(
                                out=pss[:, :ng * HC], lhsT=ident[:],
                                rhs=xlv, start=False, stop=True)
                            if DBG and li == 0 and t == 0 and pi == 0 and gg == 0:
                                s0t = smp.tile([P, 4 * HC], dt.float32, tag="s0t",
                                               name="s0t")
                                nc.vector.tensor_copy(s0t[:, :ng * HC],
                                                      pss[:, :ng * HC])
                                nc.sync.dma_start(t_dbg["s0"][:, :ng * HC],
                                                  s0t[:, :ng * HC])
                            nc.scalar.activation(u[:, gg * HC:(gg + ng) * HC],
                                                 pss[:, :ng * HC], AF.Relu)
                        if DBG and li == 0 and t == 0 and pi == 0:
                            nc.sync.dma_start(t_dbg["u0"][:], u[:, :4 * HC])
                            nc.sync.dma_start(t_dbg["xlg0"][:], xlg[:, :4 * HC])
                        # logits: signed strided reduces over channel runs
                        lg = smp.tile([P, 2 * PASS_W], dt.float32, tag="lg")
                        for h in range(H):
                            acc = None
                            for sgn in (+1, -1):
                                for (ro, rl) in L["runs"][(h, sgn)]:
                                    red = smp.tile([P, PASS_W], dt.float32, tag="red")
                                    uv = u[:, :nw * HC].rearrange(
                                        "p (j c) -> p j c", c=HC)[:, :, h * C + ro:
                                                                  h * C + ro + rl]
                                    nc.vector.reduce_sum(red[:, :nw], uv,
                                                         axis=mybir.AxisListType.X)
                                    dstv = lg[:, h * PASS_W:h * PASS_W + nw]
                                    if acc is None:
                                        nc.vector.tensor_copy(dstv, red[:, :nw])
                                        acc = True
                                    else:
                                        nc.vector.tensor_tensor(
                                            out=dstv, in0=dstv, in1=red[:, :nw],
                                            op=OP.add if sgn > 0 else OP.subtract)
                        # linear term: lin = axl[src] + axr[dst] + aef  (x0.2
                        # already folded on host); logits = 0.8*red + lin
                        lin_t = smp.tile([P, 2 * PASS_W], dt.float32, tag="lin_t")
                        aef_t = smp.tile([P, 2 * PASS_W], dt.float32, tag="aef_t")
                        nc.sync.dma_start(
                            aef_t[:, :nw * 2],
                            t_aef[li][:, (off[t] + j0) * 2:(off[t] + j0 + nw) * 2])
                        axl_v = xlg[:, :nw * W2].rearrange(
                            "p (j w) -> p j w", w=W2)[:, :, HC:HC + 2]                             .rearrange("p j h -> p h j")
                        aef_v = aef_t[:, :nw * 2].rearrange(
                            "p (j h) -> p h j", h=2)
                        lin_v = lin_t[:].rearrange("p (h j) -> p h j", h=2)[:, :, :nw]
                        nc.vector.tensor_tensor(out=lin_v, in0=axl_v, in1=aef_v,
                                                op=OP.add)
                        axr_v = xr_sb[:, HC:HC + 2].rearrange(
                            "p (h o) -> p h o", o=1).to_broadcast([P, 2, nw])
                        nc.vector.tensor_tensor(out=lin_v, in0=lin_v, in1=axr_v,
                                                op=OP.add)
                        lgv0 = lg[:].rearrange("p (h j) -> p h j", h=2)[:, :, :nw]
                        nc.vector.scalar_tensor_tensor(
                            out=lgv0, in0=lgv0, scalar=0.8, in1=lin_v,
                            op0=OP.mult, op1=OP.add)
                        if DBG and li == 0 and t == 0 and pi == 0:
                            nc.sync.dma_start(t_dbg["lg"][:, :2 * PASS_W], lg[:])
                        # mask -> exp
                        pex = smp.tile([P, 2 * PASS_W], dt.float32, tag="pex")
                        mk = msk_sb[:, off[t] + j0:off[t] + j0 + nw]
                        mk2 = mk.rearrange("p (o j) -> p o j", o=1).to_broadcast(
                            [P, 2, nw])
                        lgv = lg[:].rearrange("p (h j) -> p h j", h=2)[:, :, :nw]
                        nc.vector.scalar_tensor_tensor(
                            out=lgv, in0=lgv, scalar=MBIG, in1=mk2,
                            op0=OP.add, op1=OP.mult)
                        pexv = pex[:].rearrange("p (h j) -> p h j", h=2)[:, :, :nw]
                        nc.scalar.activation(pexv, lgv, AF.Exp, bias=negM[:])
                        if DBG and li == 0 and t == 0 and pi == 0:
                            nc.sync.dma_start(t_dbg["pex"][:, :2 * PASS_W], pex[:])
                        # denom accumulate
                        red2 = smp.tile([P, 2], dt.float32, tag="red2")
                        nc.vector.reduce_sum(red2[:], pexv, axis=mybir.AxisListType.X)
                        if pi == 0:
                            nc.vector.tensor_copy(den[:], red2[:])
                        else:
                            nc.vector.tensor_tensor(out=den[:], in0=den[:],
                                                    in1=red2[:], op=OP.add)
                        # w = p * xlg  (in place), then aggregate per slot
                        pbc = pex[:].rearrange("p (h j) -> p h j", h=2)[:, :, :nw]
                        pbc = pbc.rearrange("p h j -> p j h")
                        pbc = pbc.rearrange("p j (h o) -> p j h o", o=1) \
                                 .to_broadcast([P, nw, 2, C])
                        xv4 = xlg[:, :nw * W2].rearrange(
                            "p (j w) -> p j w", w=W2)[:, :, :HC].rearrange(
                            "p j (h c) -> p j h c", h=2)
                        nc.vector.tensor_tensor(out=xv4, in0=xv4, in1=pbc,
                                                op=OP.mult)
                        for j in range(nw):
                            nc.tensor.matmul(out=pso[:],
                                             lhsT=ident[:],
                                             rhs=xlg[:, j * W2:j * W2 + HC],
                                             start=(pi == 0 and j == 0),
                                             stop=(pi == npass - 1 and j == nw - 1))
                    # finalize tile
                    if DBG and li == 0 and t == 0:
                        nc.sync.dma_start(t_dbg["den"][:], den[:])
                        dps = smp.tile([P, HC], dt.float32, tag="dps", name="dps")
                        nc.scalar.activation(dps[:], pso[:], AF.Copy)
                        nc.sync.dma_start(t_dbg["pso"][:], dps[:])
                    nc.vector.tensor_scalar_max(den[:], den[:], 1e-16)
                    rden = smp.tile([P, 2], dt.float32, tag="rden")
                    nc.vector.reciprocal(rden[:], den[:])
                    o1 = hhp.tile([P, HC], dt.float32, tag="o1")
                    nc.vector.tensor_tensor(out=o1[:], in0=pso[:],
                                            in1=W[(li, "ra")][:], op=OP.mult)
                    rdb = rden[:].rearrange("p (h o) -> p h o", o=1) \
                                 .to_broadcast([P, 2, C])
                    o1v = o1[:].rearrange("p (h c) -> p h c", h=2)
                    nc.vector.tensor_tensor(out=o1v, in0=o1v, in1=rdb, op=OP.mult)
                    oh = hhp.tile([P, 64], dt.float32, tag="oh")
                    if li == 0:
                        nc.vector.tensor_tensor(out=oh[:], in0=o1[:],
                                                in1=W[(li, "bi")][:], op=OP.add)
                    else:
                        # mean over heads then + bias
                        nc.vector.tensor_tensor(out=oh[:], in0=o1[:, :C],
                                                in1=o1[:, C:], op=OP.add)
                        nc.vector.tensor_scalar_mul(oh[:], oh[:], 0.5)
                        nc.vector.tensor_tensor(out=oh[:], in0=oh[:],
                                                in1=W[(li, "bi")][:], op=OP.add)
                    if last:
                        nc.sync.dma_start(t_out[t * P:(t + 1) * P, :], oh[:])
                    else:
                        hp = hhp.tile([P, 64], dt.float32, tag="hp")
                        nc.scalar.activation(hp[:], oh[:], AF.Lrelu, alpha=NEG_ACT)
                        # transpose -> actT_l
                        pstr = ps_r.tile([64, P], dt.float32, tag="pstr")
                        nc.tensor.transpose(out=pstr[:], in_=hp[:, :64],
                                            identity=ident[:])
                        trs = hhp.tile([64, P], dt.float32, tag="trs")
                        nc.scalar.activation(trs[:], pstr[:], AF.Copy)
                        nc.sync.dma_start(actT_l[0:64, t * P:(t + 1) * P], trs[:])
                if not last:
                    for q0 in range(0, npc, 512):
                        qn = min(512, npc - q0)
                        nc.sync.dma_start(actT_l[64:65, q0:q0 + qn],
                                          ones[0:1, :qn])
                    nc.gpsimd.collective_compute(
                        "AllGather", mybir.AluOpType.bypass,
                        replica_groups=[list(range(NCORES))],
                        ins=[actT_l.opt()], outs=[actT_a.opt()])
                    actT_l_prev, actT_a_prev = actT_l, actT_a
    return nc


def _run(x, edge_index, edge_attr, layers):
    import os
    import concourse.bacc as bacc
    from concourse.bass_utils import run_bass_kernel_spmd

    N, F0 = x.shape
    g = _plan_graph(edge_index, N)
    # per-core eaT in slot-column order
    eaT = edge_attr.astype(np.float32).T               # [10, E]
    for c in range(NCORES):
        m = g["core"] == c
        g["EAP"][c][:, g["eacol"][m]] = eaT[:, m]

    in_perm = None
    LY = []
    for li, Lw in enumerate(layers):
        Lp = _prep_layer(*Lw, in_perm)
        LY.append(Lp)
        if li == 0:
            in_perm = np.concatenate([Lp["cho"], Lp["C"] + Lp["cho"]])
        else:
            in_perm = Lp["cho"]
    out_perm = in_perm

    xp = np.zeros((g["N_pad"], F0), np.float32)
    xp[g["perm"][:N]] = x
    xT = np.vstack([xp.T, np.ones((1, g["N_pad"]), np.float32)])

    nc = bacc.Bacc("TRN2", target_bir_lowering=False, num_devices=NCORES)
    _build(nc, g, LY, F0)
    nc.compile()

    npc = g["npc"]
    # per-layer aef in slot order: AEF[c][p, col*2 + h]
    colpos = g["eacol"] // P
    prow = g["eacol"] % P
    AEF = []
    for li, L in enumerate(LY):
        aef_e = (edge_attr @ L["Wea"]).astype(np.float32)      # [E, 2]
        A = np.zeros((NCORES, P, g["S"] * 2), np.float32)
        for c in range(NCORES):
            m = g["core"] == c
            A[c, prow[m], colpos[m] * 2] = aef_e[m, 0]
            A[c, prow[m], colpos[m] * 2 + 1] = aef_e[m, 1]
        AEF.append(A)
    in_maps = []
    for c in range(NCORES):
        m = {"xT": xT, "xTo": np.ascontiguousarray(xT[:, c * npc:(c + 1) * npc]),
             "idx": g["IDX"][c], "msk": g["MASK"][c], "eaT": g["EAP"][c]}
        for li, L in enumerate(LY):
            m[f"Wl{li}"] = L["Wl"]; m[f"Wr{li}"] = L["Wr"]; m[f"We{li}"] = L["We"]
            m[f"ra{li}"] = L["recipatt"]; m[f"bi{li}"] = L["bias_row"]
            m[f"aef{li}"] = AEF[li][c]
        in_maps.append(m)

    _tenv = os.environ.get("KBENCH_TRACE", "")
    _tkw = {}
    if _tenv:
        _cores = [int(t) for t in _tenv.split(",") if t.strip().isdigit()]
        _tkw = dict(trace=True, trace_cores=_cores or [0])
    import time as _time
    _t0 = _time.perf_counter()
    res = run_bass_kernel_spmd(nc, in_maps, list(range(NCORES)))
    _w1 = _time.perf_counter() - _t0
    _t0 = _time.perf_counter()
    res = run_bass_kernel_spmd(nc, in_maps, list(range(NCORES)), **_tkw)
    _w2 = _time.perf_counter() - _t0
    global _LAST_RES, _LAST_G, _LAST_LY, _LAST_WALL
    _LAST_RES, _LAST_G, _LAST_LY = res, g, LY
    _LAST_WALL = (_w1, _w2)
    h_new = np.concatenate([res.results[c]["o_h"] for c in range(NCORES)], axis=0)
    outp = h_new[g["perm"][:N]]                        # back to original rows
    invc = np.argsort(out_perm)                        # final column unpermute
    return np.ascontiguousarray(outp[:, invc])


def kernel(x, edge_index, edge_attr, Wl0, bl0, Wr0, br0, We0, att0, bias0,
           Wl1, bl1, Wr1, br1, We1, att1, bias1):
    x = np.asarray(x, np.float32)
    layers = [
        (np.asarray(Wl0, np.float32), np.asarray(bl0, np.float32),
         np.asarray(Wr0, np.float32), np.asarray(br0, np.float32),
         np.asarray(We0, np.float32), np.asarray(att0, np.float32),
         np.asarray(bias0, np.float32)),
    ]
    for i in range(2):
        layers.append(
            (np.asarray(Wl1[i], np.float32), np.asarray(bl1[i], np.float32),
             np.asarray(Wr1[i], np.float32), np.asarray(br1[i], np.float32),
             np.asarray(We1[i], np.float32), np.asarray(att1[i], np.float32),
             np.asarray(bias1[i], np.float32)))
    return _run(x, np.asarray(edge_index), np.asarray(edge_attr, np.float32),
                layers)

